# revision 2
# baseline (speedup 1.0000x reference)
"""Trainium2 Bass kernel for BasicRecurrentEntityEncoder (v3).

Math (per batch b, entity k, step t):
  enc[b,t,:]  = sum_l mask[b,t,l] * emb[prgrph[b,t,l]] * posmask[l,:]
  g           = sigmoid((h+keys)·s + maskbias)    (mask folded into ks bias)
  h_tilda     = sigmoid(h@U + keys@V + s@W)
  h           = normalize(h + g*h_tilda)          (exact when g=0: h is 0 or unit)

Sharding: data-parallel over batch, 8 paragraphs per core.

Layouts (BL=8 local paragraphs, K=64, D=128 -> 512 state cols, c=b*64+k):
  feature-major: hT [d=128, c]                      (PE stationary / gate dots)
  layout-B:      chunk j=c>>7, partition p=c&127    (everything else)
                 so b = 2j + (p>>6), k = p&63

v3: pre-activation computed directly in layout-B per chunk j:
      pA[c,d'] = B2.T @ sW(t)  +  I.T @ kVB_j  +  hT_j.T @ U
    where sW[8t+b,:] = enc(b,t)@W and kVB_j = keysT_j.T@V are precomputed on
    device. Kills the forward transposes; the gate matmul shares its
    stationary (hT_j) with the U-term. Sigmoid lands already in layout-B.
    Squares: group 0 on ACT (Square+accum_out), group 1 on DVE (balance).
    2 column groups -> two interleaved recurrence chains.
"""
import numpy as np

import concourse.bass as bass
import concourse.bacc as bacc
import concourse.tile as tile
from concourse import mybir
from concourse.bass_utils import run_bass_kernel_spmd

F32 = mybir.dt.float32
I32 = mybir.dt.int32
AF = mybir.ActivationFunctionType
ALU = mybir.AluOpType

B, T, L, D, K, V = 64, 128, 32, 128, 64, 50000
NCORES = 8
BL = B // NCORES              # 8 paragraphs per core
COLS = BL * K                 # 512 state columns per core
NJ = COLS // 128              # 4 layout-B chunks
NG = 2                        # column groups (2 chunks each)
GC = COLS // NG               # 256 cols per group
WORDS = BL * T * L            # 32768 gathered words per core
CHUNKS = WORDS // 128         # 256
G = 8                         # chunks per gather tile
NGI = CHUNKS // G             # 32 gather tiles
MAGIC = 0x5F3759DF
NR_ITERS = 1

_cache = {}

DBG_PHASE1 = True
DBG_SCAN_T = T


def _build_nc():
    nc = bacc.Bacc(None, target_bir_lowering=False)

    emb_t = nc.dram_tensor("emb", [V, D], F32, kind="ExternalInput")
    gidx_t = nc.dram_tensor("gidx", [NGI, 128, G], I32, kind="ExternalInput")
    mo_t = nc.dram_tensor("maskones", [NGI, 128, G, 4], F32, kind="ExternalInput")
    posrep_t = nc.dram_tensor("posrep", [128, 128], F32, kind="ExternalInput")
    keysT_t = nc.dram_tensor("keysT", [128, COLS], F32, kind="ExternalInput")
    U_t = nc.dram_tensor("Uw", [D, D], F32, kind="ExternalInput")
    V_t = nc.dram_tensor("Vw", [D, D], F32, kind="ExternalInput")
    W_t = nc.dram_tensor("Ww", [D, D], F32, kind="ExternalInput")
    ksb_t = nc.dram_tensor("ksbias", [128, 4 * T], F32, kind="ExternalInput")
    oh_t = nc.dram_tensor("oh32g", [128, 32], F32, kind="ExternalInput")
    ohb_t = nc.dram_tensor("ohB", [8, COLS], F32, kind="ExternalInput")
    id_t = nc.dram_tensor("ident", [128, 128], F32, kind="ExternalInput")
    z_t = nc.dram_tensor("zeros", [128, COLS], F32, kind="ExternalInput")
    out_t = nc.dram_tensor("h_out", [BL, K, D], F32, kind="ExternalOutput")

    with tile.TileContext(nc) as tc:
        with tc.tile_pool(name="persist", bufs=1) as pp:
            posrep = pp.tile([128, 128], F32)
            keysT = pp.tile([128, COLS], F32)
            Uw = pp.tile([D, D], F32)
            Vw = pp.tile([D, D], F32)
            Ww = pp.tile([D, D], F32)
            oh32 = pp.tile([128, 32], F32)
            ohB = pp.tile([8, COLS], F32)
            ident = pp.tile([128, 128], F32)
            encT = pp.tile([128, T * BL], F32)      # [d, 8t+b]
            ksstm = pp.tile([128, 4 * T], F32)      # [p, 4t+j] k.enc + maskbias
            kVB = pp.tile([128, COLS], F32)         # [p, 128j+d'] keys@V in B
            sw8 = pp.tile([8, T * 128], F32)        # [b, 128t+d'] enc(b,t)@W
            nc.sync.dma_start(out=posrep, in_=posrep_t[:, :])
            nc.sync.dma_start(out=keysT, in_=keysT_t[:, :])
            nc.sync.dma_start(out=Uw, in_=U_t[:, :])
            nc.sync.dma_start(out=Vw, in_=V_t[:, :])
            nc.sync.dma_start(out=Ww, in_=W_t[:, :])
            nc.sync.dma_start(out=oh32, in_=oh_t[:, :])
            nc.sync.dma_start(out=ohB, in_=ohb_t[:, :])
            nc.sync.dma_start(out=ident, in_=id_t[:, :])
            nc.sync.dma_start(out=ksstm, in_=ksb_t[:, :])

            # ---------------- Phase 1: gather + sentence encoder ----------
            with tc.tile_pool(name="p1sb", bufs=3) as p1, \
                 tc.tile_pool(name="p1w", bufs=3) as p1w, \
                 tc.tile_pool(name="p1ps", bufs=2, space="PSUM") as p1ps:
                penc = None
                for n in range(NGI if DBG_PHASE1 else 0):
                    idx = p1.tile([128, G], I32, tag="idx")
                    nc.sync.dma_start(out=idx, in_=gidx_t[n, :, :])
                    mo = p1.tile([128, G, 4], F32, tag="mo")
                    nc.sync.dma_start(out=mo, in_=mo_t[n, :, :, :])
                    embg = p1.tile([128, G, 128], F32, tag="embg")
                    for g in range(G):
                        nc.gpsimd.indirect_dma_start(
                            out=embg[:, g, :], out_offset=None, in_=emb_t[:, :],
                            in_offset=bass.IndirectOffsetOnAxis(
                                ap=idx[:, g:g + 1], axis=0))
                    for g in range(G):
                        ch = n * G + g
                        if ch % 32 == 0:
                            penc = p1ps.tile([128, 128], F32, tag="penc")
                        wt = p1w.tile([128, 128], F32, tag="wt")
                        nc.vector.tensor_tensor(
                            out=wt, in0=embg[:, g, :], in1=posrep, op=ALU.mult)
                        nc.tensor.matmul(
                            out=penc[:, (ch % 32) * 4:(ch % 32) * 4 + 4],
                            lhsT=wt, rhs=mo[:, g, :], start=True, stop=True)
                        if ch % 32 == 31:
                            nc.scalar.copy(
                                out=encT[:, (ch // 32) * 128:(ch // 32) * 128 + 128],
                                in_=penc)

            # -------- Phase 1.5: ks table, sW table, kVB (blocked by tb) ---
            with tc.tile_pool(name="ksps", bufs=2, space="PSUM") as ksps, \
                 tc.tile_pool(name="swps", bufs=2, space="PSUM") as swps, \
                 tc.tile_pool(name="swps8", bufs=1, space="PSUM") as swps8:
                # kVB_j = keysT_j.T @ V  (once)
                for j in range(NJ):
                    pkv = swps.tile([128, 128], F32, tag="pkv")
                    nc.tensor.matmul(out=pkv,
                                     lhsT=keysT[:, 128 * j:128 * (j + 1)],
                                     rhs=Vw, start=True, stop=True)
                    nc.scalar.copy(out=kVB[:, 128 * j:128 * (j + 1)], in_=pkv)
                for tb in range(8 if DBG_PHASE1 else 0):
                    # sW block: sw8[b, 128t+d'] for t in [16tb, 16tb+16)
                    psw8 = swps8.tile([8, 16 * 128], F32, tag="psw8")
                    for tt in range(16):
                        t = 16 * tb + tt
                        nc.tensor.matmul(
                            out=psw8[:, 128 * tt:128 * (tt + 1)],
                            lhsT=encT[:, 8 * t:8 * t + 8],
                            rhs=Ww, start=True, stop=True)
                    nc.vector.tensor_copy(
                        out=sw8[:, 2048 * tb:2048 * (tb + 1)], in_=psw8)
                    # ks block
                    for b in range(BL):
                        psk = ksps.tile([64, 16], F32, tag="psk")
                        encb = bass.AP(
                            tensor=encT.tensor,
                            offset=encT.offset + 8 * 16 * tb + b,
                            ap=[encT.ap[0], [BL, 16]])
                        nc.tensor.matmul(out=psk,
                                         lhsT=keysT[:, b * 64:(b + 1) * 64],
                                         rhs=encb, start=True, stop=True)
                        dst = ksstm[(b & 1) * 64:(b & 1) * 64 + 64,
                                    64 * tb + (b >> 1):64 * (tb + 1):4]
                        nc.vector.tensor_tensor(out=dst, in0=dst, in1=psk,
                                                op=ALU.add)

            # ---------------- Phase 2: the scan ---------------------------
            with tc.tile_pool(name="st", bufs=2) as stp, \
                 tc.tile_pool(name="sm", bufs=3) as smp, \
                 tc.tile_pool(name="scr", bufs=2) as scrp, \
                 tc.tile_pool(name="psA", bufs=2, space="PSUM") as psA, \
                 tc.tile_pool(name="psG", bufs=1, space="PSUM") as psG, \
                 tc.tile_pool(name="psT", bufs=1, space="PSUM") as psT:
                hT = []
                hB = []
                for g in range(NG):
                    ht = stp.tile([128, GC], F32, tag=f"hT{g}")
                    hb = stp.tile([128, GC], F32, tag=f"hB{g}")
                    nc.sync.dma_start(out=ht, in_=z_t[:, g * GC:(g + 1) * GC])
                    nc.sync.dma_start(out=hb, in_=z_t[:, g * GC:(g + 1) * GC])
                    hT.append(ht)
                    hB.append(hb)

                for t in range(DBG_SCAN_T):
                    s_sl = encT[:, 8 * t:8 * t + 8]
                    tb = t // 16
                    # --- pre-activation in layout-B, per chunk ---
                    pAs = []
                    pGs = []
                    for g in range(NG):
                        pA = psA.tile([128, GC], F32, tag=f"pA{g}")
                        pAs.append(pA)
                        pG = psG.tile([128, 16], F32, tag=f"pG{g}")
                        pGs.append(pG)
                    # V-term opens the whole-tile accumulation group
                    # (one PSUM bank = one zero region: exactly one group
                    # may be pending per bank, so the opener must span it)
                    for g in range(NG):
                        nc.tensor.matmul(
                            out=pAs[g],
                            lhsT=ident, rhs=kVB[:, GC * g:GC * (g + 1)],
                            start=True, stop=False)
                    # W-term: out[c,d'] = sw8[b(c), 128t+d'] via one-hot lhsT
                    for g in range(NG):
                        for jl in range(2):
                            j = 2 * g + jl
                            nc.tensor.matmul(
                                out=pAs[g][:, 128 * jl:128 * (jl + 1)],
                                lhsT=ohB[:, 128 * j:128 * (j + 1)],
                                rhs=sw8[:, 128 * t:128 * (t + 1)],
                                start=False, stop=False)
                    # U-term + gate dots (stationary hT_j shared)
                    for g in range(NG):
                        for jl in range(2):
                            hTj = hT[g][:, 128 * jl:128 * (jl + 1)]
                            nc.tensor.matmul(
                                out=pAs[g][:, 128 * jl:128 * (jl + 1)],
                                lhsT=hTj, rhs=Uw, start=False,
                                stop=(jl == 1))
                            nc.tensor.matmul(
                                out=pGs[g][:, 8 * jl:8 * jl + 8],
                                lhsT=hTj, rhs=s_sl, start=True, stop=True)

                    for g in range(NG):
                        # h_tilda (already layout-B)
                        htB = scrp.tile([128, GC], F32, tag=f"htB{g}")
                        nc.scalar.activation(out=htB, in_=pAs[g],
                                             func=AF.Sigmoid)

                        # gate select + sigmoid
                        gsel = smp.tile([128, 16], F32, tag=f"gsel{g}")
                        nc.vector.tensor_tensor(out=gsel, in0=pGs[g],
                                                in1=oh32[:, 16 * g:16 * g + 16],
                                                op=ALU.mult)
                        graw = smp.tile([128, 2], F32, tag=f"graw{g}")
                        nc.vector.tensor_reduce(
                            out=graw,
                            in_=gsel.rearrange("p (a b) -> p a b", b=8),
                            axis=mybir.AxisListType.X, op=ALU.add)
                        gks = smp.tile([128, 2], F32, tag=f"gks{g}")
                        nc.vector.tensor_tensor(
                            out=gks, in0=graw,
                            in1=ksstm[:, 4 * t + 2 * g:4 * t + 2 * g + 2],
                            op=ALU.add)
                        gm = smp.tile([128, 2], F32, tag=f"gm{g}")
                        nc.scalar.activation(out=gm, in_=gks, func=AF.Sigmoid)

                        # hn = h + g*h_tilda  (layout B)
                        hnB = scrp.tile([128, GC], F32, tag=f"hnB{g}")
                        for jl in range(2):
                            nc.vector.scalar_tensor_tensor(
                                out=hnB[:, 128 * jl:128 * (jl + 1)],
                                in0=htB[:, 128 * jl:128 * (jl + 1)],
                                scalar=gm[:, jl:jl + 1],
                                in1=hB[g][:, 128 * jl:128 * (jl + 1)],
                                op0=ALU.mult, op1=ALU.add)

                        # ss = sum_d hn^2 : group 0 on ACT, group 1 on DVE
                        ss = smp.tile([128, 2], F32, tag=f"ss{g}")
                        if g == 0:
                            for jl in range(2):
                                dump = scrp.tile([128, 128], F32,
                                                 tag=f"dump{g}")
                                nc.scalar.activation(
                                    out=dump,
                                    in_=hnB[:, 128 * jl:128 * (jl + 1)],
                                    func=AF.Square,
                                    accum_out=ss[:, jl:jl + 1])
                        else:
                            sq = scrp.tile([128, GC], F32, tag=f"sq{g}")
                            nc.vector.tensor_tensor(out=sq, in0=hnB, in1=hnB,
                                                    op=ALU.mult)
                            nc.vector.tensor_reduce(
                                out=ss,
                                in_=sq.rearrange("p (a b) -> p a b", b=128),
                                axis=mybir.AxisListType.X, op=ALU.add)

                        if NR_ITERS >= 2:
                            ssc = smp.tile([128, 2], F32, tag=f"ssc{g}")
                            nc.vector.tensor_scalar(out=ssc, in0=ss,
                                                    scalar1=1e-12,
                                                    scalar2=None, op0=ALU.max)
                        else:
                            # NR=1 tolerates ss=0 (0 * huge = 0), skip clamp
                            ssc = ss
                        # rsqrt: magic seed (int32 domain) + NR iterations
                        seed = smp.tile([128, 2], I32, tag=f"seed{g}")
                        nc.vector.tensor_scalar(out=seed, in0=ssc.bitcast(I32),
                                                scalar1=-0.5,
                                                scalar2=float(MAGIC),
                                                op0=ALU.mult, op1=ALU.add)
                        y = seed.bitcast(F32)
                        for it in range(NR_ITERS):
                            t1 = smp.tile([128, 2], F32, tag=f"t1{g}")
                            t2 = smp.tile([128, 2], F32, tag=f"t2{g}")
                            t3 = smp.tile([128, 2], F32, tag=f"t3{g}")
                            yn = smp.tile([128, 2], F32, tag=f"yn{g}")
                            nc.vector.tensor_tensor(out=t1, in0=y, in1=y,
                                                    op=ALU.mult)
                            nc.vector.tensor_tensor(out=t2, in0=t1, in1=ssc,
                                                    op=ALU.mult)
                            nc.vector.tensor_scalar(out=t3, in0=t2, scalar1=-0.5,
                                                    scalar2=1.5, op0=ALU.mult,
                                                    op1=ALU.add)
                            nc.vector.tensor_tensor(out=yn, in0=t3, in1=y,
                                                    op=ALU.mult)
                            y = yn

                        # h' = hn * inv (layout B)
                        hBn = stp.tile([128, GC], F32, tag=f"hB{g}")
                        for jl in range(2):
                            nc.vector.tensor_scalar(
                                out=hBn[:, 128 * jl:128 * (jl + 1)],
                                in0=hnB[:, 128 * jl:128 * (jl + 1)],
                                scalar1=y[:, jl:jl + 1], scalar2=None,
                                op0=ALU.mult)
                        hB[g] = hBn

                        if t < DBG_SCAN_T - 1:
                            # transpose back to feature-major for next step
                            pH = psT.tile([128, GC], F32, tag=f"tr{g}")
                            for jl in range(2):
                                nc.tensor.transpose(
                                    out=pH[:, 128 * jl:128 * (jl + 1)],
                                    in_=hBn[:, 128 * jl:128 * (jl + 1)],
                                    identity=ident)
                            hTn = stp.tile([128, GC], F32, tag=f"hT{g}")
                            nc.scalar.copy(out=hTn, in_=pH)
                            hT[g] = hTn

                # -------- output: h[b,k,:] = hB[b>>2][(b&1)*64+k, 128*((b>>1)&1)+:]
                for b in range(BL):
                    src = hB[b >> 2][(b & 1) * 64:(b & 1) * 64 + 64,
                                     128 * ((b >> 1) & 1):128 * ((b >> 1) & 1) + 128]
                    nc.sync.dma_start(out=out_t[b, :, :], in_=src)
    nc.compile()
    return nc


def _prep_core(core, prgrph, prgrph_mask, embedding_matrix, positional_mask,
               Uw, Vw, Ww, keys):
    b0 = core * BL
    pr = prgrph[b0:b0 + BL]          # [8, T, L]
    pm = prgrph_mask[b0:b0 + BL]
    ky = keys[b0:b0 + BL]            # [8, K, D]

    idx_core = np.ascontiguousarray(pr.transpose(1, 0, 2)).reshape(-1)  # (t,b,l)
    gidx = np.ascontiguousarray(
        idx_core.reshape(NGI, G, 128).transpose(0, 2, 1)).astype(np.int32)

    maskf = pm.transpose(1, 0, 2).reshape(-1).astype(np.float32)
    mw = maskf.reshape(CHUNKS, 4, 32)
    mo = np.zeros((CHUNKS, 128, 4), dtype=np.float32)
    for jj in range(4):
        mo[:, jj * 32:(jj + 1) * 32, jj] = mw[:, jj, :]
    mo = np.ascontiguousarray(
        mo.reshape(NGI, G, 128, 4).transpose(0, 2, 1, 3))

    posrep = np.ascontiguousarray(np.tile(positional_mask, (4, 1))).astype(np.float32)
    keysT = np.ascontiguousarray(ky.transpose(2, 0, 1).reshape(D, COLS))

    # layout-B: partition p, chunk j -> b = 2j + (p>>6)
    p_ar = np.arange(128)
    j_ar = np.arange(4)
    b_of = 2 * j_ar[None, :] + (p_ar[:, None] >> 6)          # [128, 4]
    msent = pm.any(axis=2).astype(np.float32)                # [8, T]
    # ks bias table [p, 4t+j]: -3e4 where the sentence is masked out
    ksbias = np.ascontiguousarray(
        (-30000.0 * (1.0 - msent[b_of])).transpose(0, 2, 1).reshape(128, 4 * T)
    ).astype(np.float32)
    # group-local one-hot: col 16g + 8jl + b  ->  1 iff b == b_of(p, 2g+jl)
    oh32g = np.zeros((128, 32), dtype=np.float32)
    for g in range(NG):
        for jl in range(2):
            j = 2 * g + jl
            oh32g[p_ar, 16 * g + 8 * jl + b_of[:, j]] = 1.0
    ident = np.eye(128, dtype=np.float32)
    # ohB[b, c] = 1 iff b == b_of(c) = 2*(c>>7) + ((c&127)>>6)
    c_ar = np.arange(COLS)
    ohBm = np.zeros((8, COLS), dtype=np.float32)
    ohBm[2 * (c_ar >> 7) + ((c_ar & 127) >> 6), c_ar] = 1.0
    zeros = np.zeros((128, COLS), dtype=np.float32)

    return {
        "emb": np.ascontiguousarray(embedding_matrix.astype(np.float32)),
        "gidx": gidx, "maskones": mo, "posrep": posrep,
        "keysT": keysT,
        "Uw": np.ascontiguousarray(Uw.astype(np.float32)),
        "Vw": np.ascontiguousarray(Vw.astype(np.float32)),
        "Ww": np.ascontiguousarray(Ww.astype(np.float32)),
        "ksbias": ksbias, "oh32g": oh32g, "ident": ident,
        "ohB": ohBm, "zeros": zeros,
    }


def kernel(prgrph, prgrph_mask, embedding_matrix, positional_mask,
           Uw, Vw, Ww, keys, _trace=False):
    prgrph = np.asarray(prgrph)
    prgrph_mask = np.asarray(prgrph_mask)
    embedding_matrix = np.asarray(embedding_matrix, dtype=np.float32)
    positional_mask = np.asarray(positional_mask, dtype=np.float32)
    Uw = np.asarray(Uw, dtype=np.float32)
    Vw = np.asarray(Vw, dtype=np.float32)
    Ww = np.asarray(Ww, dtype=np.float32)
    keys = np.asarray(keys, dtype=np.float32)

    if "nc" not in _cache:
        _cache["nc"] = _build_nc()
    nc = _cache["nc"]

    in_maps = [_prep_core(c, prgrph, prgrph_mask, embedding_matrix,
                          positional_mask, Uw, Vw, Ww, keys)
               for c in range(NCORES)]
    res = run_bass_kernel_spmd(nc, in_maps, core_ids=list(range(NCORES)),
                               trace=_trace)
    outs = [np.asarray(r["h_out"]).reshape(BL, K, D) for r in res.results]
    full = np.concatenate(outs, axis=0)
    if _trace:
        kernel.last_results = res
    return full
